# revision 9
# baseline (speedup 1.0000x reference)
"""Trainium2 Bass kernel for MoE routing (2-layer expert MLP + softmax).

Strategy: expert-parallel across the 8 NeuronCores. The reference computes
all 8 experts for every sample and then gathers the one selected by
`domain`; mathematically only the selected expert's MLP matters per sample.
The host groups samples by expert (argsort of `domain`), core e receives
only the ~B/8 samples routed to expert e (padded to a uniform per-core
capacity so all cores run the same SPMD program) plus expert e's weights.
Each core runs a dense 2-layer MLP + softmax in a transposed layout:

    hT[f2, n]  = relu(W1[:, f2].T @ xT[:, n] + b1[f2])   (PE bf16 + ACT)
    lT[c, n]   = W2[:, c].T @ hT[:, n]                   (PE bf16)
    expT       = exp(lT + b2)                            (ACT)
    sT[c, n]   = ones[C,C].T @ expT                      (PE: partition sum,
                                                          pre-broadcast to C)
    out[c, n]  = expT * (1 / sT)                         (DVE)

The MLP matmuls run in bf16 (host converts x/W1/W2; PSUM accumulation is
fp32, and the logits' ~0.3% bf16 noise is far inside the 2e-2 gate). bf16
matters not for the stream rate (fp32r already streams 1 cycle/row) but
because only 2-byte weights support a standalone LDWEIGHTS shared by many
matmuls: layer 1 runs weights-outer / batch-chunks-inner, so each of the
32 W1 tiles is loaded into the PE once per chunk group instead of once per
matmul. The tile legalizer still inserts one LDWEIGHTS per matmul, so
_dedup_ldweights() drops the (sync-free) duplicates after scheduling.
That removes ~27us of pure weight-load time from the PE stream, which the
trace showed was ~88% busy with LDWEIGHTS alone taking 43% of it.

Layer 1 is split into two chunk groups so compute on the first group's
columns overlaps the DMA of the rest of x. PSUM budget (8 banks): 4 for
the rotating layer-1 accumulators, 2 for layer-2 logits, 2 for the
softmax partition-sum. The softmax tail stays fp32r/fp32 end to end.
Inputs are pre-arranged on the host into the exact SBUF tile layouts so
every DMA descriptor is one long contiguous run per partition. The host
scatters each core's [C, cap] output back to the original row order.
"""

import math

import ml_dtypes
import numpy as np

import concourse.bacc as bacc
import concourse.bass as bass
import concourse.mybir as mybir
import concourse.tile as tile
from concourse.bass import ds
from concourse.bass_utils import run_bass_kernel_spmd

N_CORES = 8

bf16_np = np.dtype(ml_dtypes.bfloat16)

_program_cache: dict[tuple, object] = {}


def _chunk_sizes(cap: int) -> list[int]:
    """Split cap (multiple of 128) into matmul chunks of <=512 columns
    (one PSUM bank each). The first two chunks are small so the PE can
    start on chunk 0 as soon as its slice of x lands, instead of waiting
    ~5us for a full 512-column DMA."""
    if cap <= 512:
        return [cap]
    rest = cap - 512
    q, r = divmod(rest, 512)
    return [256, 256] + [512] * q + ([r] if r else [])


def _chunk_groups(n_chunks: int) -> list[list[int]]:
    """Layer-1 weight-reuse groups in DMA-arrival cascade: chunk 0 alone
    (compute starts earliest), then pairs/triples as x backfills. Group
    size is capped at 3 so the per-group PSUM accumulators (one bank per
    chunk) plus the layer-2/softmax banks fit in the 8 PSUM banks."""
    groups = [[0]]
    i = 1
    if n_chunks > 1:
        groups.append(list(range(1, min(3, n_chunks))))
        i = min(3, n_chunks)
    while i < n_chunks:
        groups.append(list(range(i, min(i + 3, n_chunks))))
        i = min(i + 3, n_chunks)
    return groups


def _dedup_ldweights(nc) -> int:
    """Drop legalizer-inserted LDWEIGHTS that reload the PE array with the
    exact weights the previous LDWEIGHTS already loaded.

    The tile legalizer splits every InstMatmult into LDWEIGHTS + MATMUL;
    consecutive matmuls that share a stationary tile then reload it for
    nothing (~128 PE cycles each). PE instructions execute in block order,
    so a load is redundant iff the previous InstLdweights in the block has
    an identical physical access pattern (memref + offset + strides +
    dtype) -- any intervening matmul on other weights carries its own
    LDWEIGHTS and breaks the chain. Only sync-free instances are removed;
    4-byte dtypes are skipped (fp32r matmuls must self-load on TRN2).
    """
    two_byte = (mybir.dt.bfloat16, mybir.dt.float16)
    removed = 0
    for blk in nc.main_func.blocks:
        last_sig = None
        keep = []
        for inst in blk.instructions:
            if isinstance(inst, mybir.InstLdweights):
                ap = inst.ins[0]
                si = inst.sync_info
                clean = si is None or (not si.on_wait and not si.on_update)
                if ap.dtype in two_byte:
                    sig = str(ap)
                    if sig == last_sig and clean:
                        removed += 1
                        continue
                    last_sig = sig
                else:
                    last_sig = None
            keep.append(inst)
        del blk.instructions[:]
        for inst in keep:
            blk.instructions.append(inst)
    return removed


def _build_program(cap: int, F1: int, F2: int, C: int):
    """Build the per-core SPMD bass program for a dense [cap, F1] -> [C, cap]
    expert MLP in transposed layout."""
    key = (cap, F1, F2, C)
    if key in _program_cache:
        return _program_cache[key]

    assert F1 % 128 == 0 and F2 % 128 == 0 and cap % 128 == 0
    K1 = F1 // 128  # contraction tiles for layer 1
    M1 = F2 // 128  # output partition tiles for layer 1
    K2 = F2 // 128  # contraction tiles for layer 2
    assert C <= 128

    f32 = mybir.dt.float32
    f32r = mybir.dt.float32r
    bf16 = mybir.dt.bfloat16
    nc = bacc.Bacc(None, target_bir_lowering=False, debug=False)

    chunks = _chunk_sizes(cap)
    groups = _chunk_groups(len(chunks))

    # All inputs arrive pre-arranged in SBUF tile layout.
    x_d = [
        nc.dram_tensor(f"xt{ci}", [128, K1, cn], bf16, kind="ExternalInput")
        for ci, cn in enumerate(chunks)
    ]
    w1_d = nc.dram_tensor("w1", [128, M1, K1, 128], bf16, kind="ExternalInput")
    b1_d = nc.dram_tensor("b1t", [128, M1], f32, kind="ExternalInput")
    w2_d = nc.dram_tensor("w2", [128, K2, C], bf16, kind="ExternalInput")
    b2_d = nc.dram_tensor("b2t", [C, 1], f32, kind="ExternalInput")
    out_d = nc.dram_tensor("outT", [C, cap], f32, kind="ExternalOutput")

    n0s = [sum(chunks[:i]) for i in range(len(chunks))]  # column offsets

    with tile.TileContext(nc) as tc:
        with (
            tc.tile_pool(name="const", bufs=1) as const_pool,
            tc.tile_pool(name="xin", bufs=len(chunks)) as x_pool,
            tc.tile_pool(name="h", bufs=M1 * len(chunks)) as h_pool,
            tc.tile_pool(name="exp", bufs=4) as e_pool,
            tc.tile_pool(name="out", bufs=4) as o_pool,
            tc.tile_pool(name="rec", bufs=4) as r_pool,
            tc.tile_pool(name="ph", bufs=4, space="PSUM") as ph_pool,
            tc.tile_pool(name="pl", bufs=2, space="PSUM") as pl_pool,
            tc.tile_pool(name="pb", bufs=2, space="PSUM") as pb_pool,
        ):
            # Weights on the ACT HWDGE ring (parallel to the x stream on the
            # SP ring). Tiny bias DMAs go FIRST: queued after the multi-MB
            # weight/x streams they complete ~10us late and relu (which
            # needs b1) stalls the whole pipeline.
            b1_sb = const_pool.tile([128, M1], f32)
            nc.scalar.dma_start(b1_sb[:], b1_d[:])
            b2_sb = const_pool.tile([C, 1], f32)
            nc.scalar.dma_start(b2_sb[:], b2_d[:])
            # w1 in M1 m-major blocks: arrival order matches the m-loop's
            # consumption order, and few DMA instructions means no HWDGE
            # semaphore-reuse serialization.
            w1_sb = const_pool.tile([128, M1, K1, 128], bf16)
            for m in range(M1):
                nc.scalar.dma_start(w1_sb[:, m, :, :], w1_d[:, m, :, :])
            w2_sb = const_pool.tile([128, K2, C], bf16)
            nc.scalar.dma_start(w2_sb[:], w2_d[:])

            # ones[C, C]: a single matmul against this computes the
            # partition-dim sum of exp AND broadcasts it back to all C
            # partitions in one shot. (memset can't write f32r; round via a
            # DVE copy.)
            ones_f32 = const_pool.tile([C, C], f32)
            nc.gpsimd.memset(ones_f32[:], 1.0)
            ones_cc = const_pool.tile([C, C], f32r)
            nc.vector.tensor_copy(ones_cc[:], ones_f32[:])

            # PE warmup: the HAM clock gate keeps the PE at 1.2 GHz until
            # it has been busy for a full ~3.4us activity window. The real
            # stream can't start until x/w DMAs land (~4us away), so burn
            # that wait on matmuls against the ones tile (no DMA deps):
            # the PE is already at 2.4 GHz when the first real matmul
            # issues, instead of running the first ~8us of layer 1 at
            # half clock.
            warm_in = const_pool.tile([C, 512], f32)
            nc.gpsimd.memset(warm_in[:], 0.0)
            for _ in range(10):
                pwarm = pb_pool.tile([C, 512], f32, tag="pb", name="pwarm")
                nc.tensor.matmul(
                    pwarm[:],
                    ones_cc[:],
                    warm_in[:].bitcast(f32r),
                    start=True,
                    stop=True,
                )

            # x for all chunks upfront on the SP ring; chunk 0 in two halves
            # so layer-1 (m=0, k=0..K1/2) can start before the rest lands.
            xt = []
            for ci, cn in enumerate(chunks):
                t = x_pool.tile([128, K1, cn], bf16, tag="xt", name="xt")
                if ci == 0:
                    nc.sync.dma_start(t[:, : K1 // 2, :], x_d[ci][:, : K1 // 2, :])
                    nc.sync.dma_start(t[:, K1 // 2 :, :], x_d[ci][:, K1 // 2 :, :])
                else:
                    nc.sync.dma_start(t[:], x_d[ci][:])
                xt.append(t)

            # ---- Layer 1: weights-outer, chunks-inner per group ---------
            # Each (m, k) weight tile is loaded once per group; the dedup
            # pass removes the redundant per-matmul reloads. Accumulation
            # groups for the chunks of a group interleave across PSUM
            # banks, which is fine on hardware (skip the sim's group
            # check).
            ht = {}  # (m, ci) -> bf16 SBUF tile
            for grp in groups:
                ph = {}
                for m in range(M1):
                    for k in range(K1):
                        for ci in grp:
                            if k == 0:
                                ph[ci] = ph_pool.tile(
                                    [128, chunks[ci]], f32, tag="ph", name="ph"
                                )
                            nc.tensor.matmul(
                                ph[ci][:],
                                w1_sb[:, m, k, :],
                                xt[ci][:, k, :],
                                start=(k == 0),
                                stop=(k == K1 - 1),
                                skip_group_check=True,
                            )
                            if k == K1 - 1:
                                hm = h_pool.tile(
                                    [128, chunks[ci]], bf16, tag="ht"
                                )
                                nc.scalar.activation(
                                    hm[:],
                                    ph[ci][:],
                                    mybir.ActivationFunctionType.Relu,
                                    bias=b1_sb[:, ds(m, 1)],
                                )
                                ht[(m, ci)] = hm

            # ---- Layer 2 + softmax, chunk-wise --------------------------
            def l2_body(ci: int):
                cn = chunks[ci]
                pl = pl_pool.tile([C, cn], f32, tag="pl")
                for k in range(K2):
                    nc.tensor.matmul(
                        pl[:],
                        w2_sb[:, k, :],
                        ht[(k, ci)][:],
                        start=(k == 0),
                        stop=(k == K2 - 1),
                    )
                expt = e_pool.tile([C, cn], f32r, tag="expt")
                nc.scalar.activation(
                    expt[:],
                    pl[:],
                    mybir.ActivationFunctionType.Exp,
                    bias=b2_sb[:, 0:1],
                )
                return expt

            def tail(expt, ci: int):
                """Softmax normalization + store for one chunk."""
                cn = chunks[ci]
                pb = pb_pool.tile([C, cn], f32, tag="pb")
                nc.tensor.matmul(pb[:], ones_cc[:], expt[:], start=True, stop=True)
                rec = r_pool.tile([C, cn], f32, tag="rec")
                nc.vector.reciprocal_approx_fast(rec[:], pb[:])
                ot = o_pool.tile([C, cn], f32, tag="ot")
                nc.vector.tensor_mul(ot[:], expt[:].bitcast(f32), rec[:])
                # GpSimd SWDGE: a store that waits on the softmax chain
                # would stall later x loads (sync ring) or relu/exp issue
                # (scalar ring); the gpsimd queue is otherwise idle.
                nc.gpsimd.dma_start(out_d[:, ds(n0s[ci], cn)], ot[:])

            # Emit tails one chunk behind the bodies so the PE stream stays
            # dense and each softmax-sum matmul overlaps the next chunk's
            # layer-2 matmuls instead of stalling on the ACT exp.
            pending = None
            for ci in range(len(chunks)):
                expt = l2_body(ci)
                if pending is not None:
                    tail(*pending)
                pending = (expt, ci)
            tail(*pending)

    # Layer-1 matmuls past the first of their (m, k, group) run should lose
    # their reload. The tile scheduler occasionally interleaves other PE
    # work and keeps a few extra loads; require at least half to confirm
    # the weight-reuse structure survived scheduling at all.
    n_removed = _dedup_ldweights(nc)
    n_l1_dups = sum(K1 * M1 * (len(grp) - 1) for grp in groups)
    assert n_removed >= n_l1_dups // 2, (n_removed, n_l1_dups)

    nc.compile()
    _program_cache[key] = nc
    return nc


def kernel(domain, x, W1, b1, W2, b2):
    domain = np.asarray(domain)
    x = np.asarray(x, dtype=np.float32)
    W1 = np.asarray(W1, dtype=np.float32)
    b1 = np.asarray(b1, dtype=np.float32)
    W2 = np.asarray(W2, dtype=np.float32)
    b2 = np.asarray(b2, dtype=np.float32)

    B, F1 = x.shape
    E, _, F2 = W1.shape
    C = W2.shape[2]
    K1 = F1 // 128
    K2 = F2 // 128
    assert E == N_CORES

    idx = [np.nonzero(domain == e)[0] for e in range(E)]
    counts = [len(i) for i in idx]
    cap = max(512, int(math.ceil(max(counts) / 128)) * 128)
    chunks = _chunk_sizes(cap)

    nc = _build_program(cap, F1, F2, C)

    x_bf = x.astype(bf16_np)
    W1_bf = W1.astype(bf16_np)
    W2_bf = W2.astype(bf16_np)

    in_maps = []
    for e in range(E):
        xT = np.zeros((F1, cap), bf16_np)
        xT[:, : counts[e]] = x_bf[idx[e]].T
        # [F1, cap] -> per-chunk [128, K1, cn] blocks (SBUF tile layout).
        xT3 = xT.reshape(K1, 128, cap)
        m = {
            "w1": np.ascontiguousarray(
                W1_bf[e].reshape(K1, 128, F2 // 128, 128).transpose(1, 2, 0, 3)
            ),
            "b1t": np.ascontiguousarray(b1[e].reshape(F2 // 128, 128).T),
            "w2": np.ascontiguousarray(
                W2_bf[e].reshape(K2, 128, C).transpose(1, 0, 2)
            ),
            "b2t": np.ascontiguousarray(b2[e].reshape(C, 1)),
        }
        n0 = 0
        for ci, cn in enumerate(chunks):
            m[f"xt{ci}"] = np.ascontiguousarray(
                xT3[:, :, n0 : n0 + cn].transpose(1, 0, 2)
            )
            n0 += cn
        in_maps.append(m)

    res = run_bass_kernel_spmd(nc, in_maps, core_ids=list(range(N_CORES)))

    out = np.empty((B, C), np.float32)
    for e in range(E):
        out[idx[e]] = res.results[e]["outT"][:, : counts[e]].T
    return out


# revision 17
# speedup vs baseline: 1.0859x; 1.0859x over previous
"""Trainium2 Bass kernel for MoE routing (2-layer expert MLP + softmax).

Strategy: expert-parallel across the 8 NeuronCores. The reference computes
all 8 experts for every sample and then gathers the one selected by
`domain`; mathematically only the selected expert's MLP matters per sample.
The host groups samples by expert (argsort of `domain`), core e receives
only the ~B/8 samples routed to expert e (padded to a uniform per-core
capacity so all cores run the same SPMD program) plus expert e's weights.
Each core runs a dense 2-layer MLP + softmax in a transposed layout:

    hT[f2, n]  = relu(W1[:, f2].T @ xT[:, n] + b1[f2])   (PE bf16 + ACT)
    lT[c, n]   = W2[:, c].T @ hT[:, n]                   (PE bf16)
    expT       = exp(lT + b2)                            (ACT)
    sT[c, n]   = ones[C,C].T @ expT                      (PE: partition sum,
                                                          pre-broadcast to C)
    out[c, n]  = expT * (1 / sT)                         (DVE)

The MLP matmuls run in bf16 (host converts x/W1/W2; PSUM accumulation is
fp32, and the logits' ~0.3% bf16 noise is far inside the 2e-2 gate). bf16
matters not for the stream rate (fp32r already streams 1 cycle/row) but
because only 2-byte weights support a standalone LDWEIGHTS shared by many
matmuls: layer 1 runs weights-outer / batch-chunks-inner, so each of the
32 W1 tiles is loaded into the PE once per chunk group instead of once per
matmul. The tile legalizer still inserts one LDWEIGHTS per matmul, so
_dedup_ldweights() drops the (sync-free) duplicates after scheduling.
That removes ~27us of pure weight-load time from the PE stream, which the
trace showed was ~88% busy with LDWEIGHTS alone taking 43% of it.

Layer 1 is split into two chunk groups so compute on the first group's
columns overlaps the DMA of the rest of x. PSUM budget (8 banks): 4 for
the rotating layer-1 accumulators, 2 for layer-2 logits, 2 for the
softmax partition-sum. The softmax tail stays fp32r/fp32 end to end.
Inputs are pre-arranged on the host into the exact SBUF tile layouts so
every DMA descriptor is one long contiguous run per partition. The host
scatters each core's [C, cap] output back to the original row order.
"""

import math

import ml_dtypes
import numpy as np

import concourse.bacc as bacc
import concourse.bass as bass
import concourse.mybir as mybir
import concourse.tile as tile
from concourse.bass import ds
from concourse.bass_utils import run_bass_kernel_spmd

N_CORES = 8

bf16_np = np.dtype(ml_dtypes.bfloat16)

_program_cache: dict[tuple, object] = {}


def _chunk_sizes(cap: int) -> list[int]:
    """Split cap (multiple of 128) into matmul chunks of <=512 columns
    (one PSUM bank each). The first two chunks are small so the PE can
    start on chunk 0 as soon as its slice of x lands, instead of waiting
    ~5us for a full 512-column DMA."""
    if cap <= 512:
        return [cap]
    rest = cap - 512
    q, r = divmod(rest, 512)
    return [256, 256] + [512] * q + ([r] if r else [])


def _chunk_groups(n_chunks: int) -> list[list[int]]:
    """Layer-1 weight-reuse groups in DMA-arrival cascade: chunk 0 alone
    (compute starts earliest), then pairs/triples as x backfills. Group
    size is capped at 3 so the per-group PSUM accumulators (one bank per
    chunk) plus the layer-2/softmax banks fit in the 8 PSUM banks."""
    groups = [[0]]
    i = 1
    if n_chunks > 1:
        groups.append(list(range(1, min(3, n_chunks))))
        i = min(3, n_chunks)
    while i < n_chunks:
        groups.append(list(range(i, min(i + 3, n_chunks))))
        i = min(i + 3, n_chunks)
    return groups


def _dedup_ldweights(nc) -> int:
    """Drop legalizer-inserted LDWEIGHTS that reload the PE array with the
    exact weights the previous LDWEIGHTS already loaded.

    The tile legalizer splits every InstMatmult into LDWEIGHTS + MATMUL;
    consecutive matmuls that share a stationary tile then reload it for
    nothing (~128 PE cycles each). PE instructions execute in block order,
    so a load is redundant iff the previous InstLdweights in the block has
    an identical physical access pattern (memref + offset + strides +
    dtype) -- any intervening matmul on other weights carries its own
    LDWEIGHTS and breaks the chain. Only sync-free instances are removed;
    4-byte dtypes are skipped (fp32r matmuls must self-load on TRN2).
    """
    two_byte = (mybir.dt.bfloat16, mybir.dt.float16)
    removed = 0
    for blk in nc.main_func.blocks:
        last_sig = None
        keep = []
        for inst in blk.instructions:
            if isinstance(inst, mybir.InstLdweights):
                ap = inst.ins[0]
                si = inst.sync_info
                clean = si is None or (not si.on_wait and not si.on_update)
                if ap.dtype in two_byte:
                    sig = str(ap)
                    if sig == last_sig and clean:
                        removed += 1
                        continue
                    last_sig = sig
                else:
                    last_sig = None
            keep.append(inst)
        del blk.instructions[:]
        for inst in keep:
            blk.instructions.append(inst)
    return removed


def _stagger_x_dmas(nc, gates: dict[str, list[str]]) -> int:
    """Delay later x-chunk DMA triggers until the chunks the PE consumes
    first have fully landed.

    All in-flight HWDGE transfers round-robin-share the 16 SDMA engines,
    so with every chunk triggered at once the first chunk only completes
    near the end of the whole stream and the PE idles for its columns.
    `gates` maps a DRAM tensor name to the DRAM tensor names whose
    transfers must complete before its trigger may generate descriptors.
    Implemented by appending `sem >= value` waits on the trigger
    instruction using the completion semaphore (and cumulative value) the
    scheduler already assigned to the gating transfer — the exact pattern
    the framework itself emits for ring-slot reuse. Purely a timing
    shaper: consumers' own data waits are untouched.
    """
    # Locate each DMA trigger reading a gated/gating DRAM tensor and the
    # cumulative value of its completion semaphore.
    names = set(gates) | {g for gs in gates.values() for g in gs}
    cum: dict[int, int] = {}
    done_val: dict[str, tuple[int, str, int]] = {}  # dram name -> (sem id, ant, cum)
    insts: dict[str, list] = {}
    for blk in nc.main_func.blocks:
        for inst in blk.instructions:
            si = inst.sync_info
            if si is None:
                continue
            for u in si.on_update:
                if u.sync_type == "semaphore" and u.update_mode == "sem-add-imm":
                    cum[u.id] = cum.get(u.id, 0) + u.update_value
                    if isinstance(inst, mybir.InstDMACopy):
                        src = inst.ins[0]
                        ref = getattr(src, "memref", "") or ""
                        if ref in names:
                            done_val[ref] = (u.id, u.ant_name, cum[u.id])
                            insts.setdefault(ref, []).append(inst)
    added = 0
    for name, deps in gates.items():
        for inst in insts.get(name, []):
            si = inst.sync_info
            waits = list(si.on_wait) if si else []
            ups = list(si.on_update) if si else []
            for dep in deps:
                if dep not in done_val:
                    continue
                sid, ant, val = done_val[dep]
                if any(w.id == sid and w.wait_value >= val for w in waits):
                    continue
                waits.append(
                    mybir.SyncWait(
                        sync_type="semaphore",
                        id=sid,
                        ant_name=ant,
                        wait_mode="sem-ge-imm",
                        wait_value=val,
                        wait_reg=None,
                    )
                )
                added += 1
            inst.sync_info = mybir.SyncInfo(on_wait=waits, on_update=ups)
    return added


def _build_program(cap: int, F1: int, F2: int, C: int):
    """Build the per-core SPMD bass program for a dense [cap, F1] -> [C, cap]
    expert MLP in transposed layout."""
    key = (cap, F1, F2, C)
    if key in _program_cache:
        return _program_cache[key]

    assert F1 % 128 == 0 and F2 % 128 == 0 and cap % 128 == 0
    K1 = F1 // 128  # contraction tiles for layer 1
    M1 = F2 // 128  # output partition tiles for layer 1
    K2 = F2 // 128  # contraction tiles for layer 2
    assert C <= 128

    f32 = mybir.dt.float32
    f32r = mybir.dt.float32r
    bf16 = mybir.dt.bfloat16
    nc = bacc.Bacc(None, target_bir_lowering=False, debug=False)

    chunks = _chunk_sizes(cap)
    groups = _chunk_groups(len(chunks))

    # All inputs arrive pre-arranged in SBUF tile layout.
    x_d = [
        nc.dram_tensor(f"xt{ci}", [128, K1, cn], bf16, kind="ExternalInput")
        for ci, cn in enumerate(chunks)
    ]
    w1_d = nc.dram_tensor("w1", [128, M1, K1, 128], bf16, kind="ExternalInput")
    b1_d = nc.dram_tensor("b1t", [128, M1], f32, kind="ExternalInput")
    w2_d = nc.dram_tensor("w2", [128, K2, C], bf16, kind="ExternalInput")
    b2_d = nc.dram_tensor("b2t", [C, 1], f32, kind="ExternalInput")
    out_d = nc.dram_tensor("outT", [C, cap], f32, kind="ExternalOutput")

    n0s = [sum(chunks[:i]) for i in range(len(chunks))]  # column offsets

    with tile.TileContext(nc) as tc:
        with (
            tc.tile_pool(name="const", bufs=1) as const_pool,
            tc.tile_pool(name="xin", bufs=len(chunks)) as x_pool,
            tc.tile_pool(name="h", bufs=M1 * len(chunks)) as h_pool,
            tc.tile_pool(name="exp", bufs=4) as e_pool,
            tc.tile_pool(name="out", bufs=4) as o_pool,
            tc.tile_pool(name="rec", bufs=4) as r_pool,
            tc.tile_pool(name="ph", bufs=4, space="PSUM") as ph_pool,
            tc.tile_pool(name="pl", bufs=2, space="PSUM") as pl_pool,
            tc.tile_pool(name="pb", bufs=2, space="PSUM") as pb_pool,
        ):
            # Weights on the ACT HWDGE ring (parallel to the x stream on the
            # SP ring). Tiny bias DMAs go FIRST: queued after the multi-MB
            # weight/x streams they complete ~10us late and relu (which
            # needs b1) stalls the whole pipeline.
            b1_sb = const_pool.tile([128, M1], f32)
            nc.scalar.dma_start(b1_sb[:], b1_d[:])
            b2_sb = const_pool.tile([C, 1], f32)
            nc.scalar.dma_start(b2_sb[:], b2_d[:])
            # w1 in M1 m-major blocks: arrival order matches the m-loop's
            # consumption order, and few DMA instructions means no HWDGE
            # semaphore-reuse serialization.
            w1_sb = const_pool.tile([128, M1, K1, 128], bf16)
            for m in range(M1):
                nc.scalar.dma_start(w1_sb[:, m, :, :], w1_d[:, m, :, :])
            w2_sb = const_pool.tile([128, K2, C], bf16)
            nc.scalar.dma_start(w2_sb[:], w2_d[:])

            # ones[C, C]: a single matmul against this computes the
            # partition-dim sum of exp AND broadcasts it back to all C
            # partitions in one shot. (memset can't write f32r; round via a
            # DVE copy.)
            ones_f32 = const_pool.tile([C, C], f32)
            nc.gpsimd.memset(ones_f32[:], 1.0)
            ones_cc = const_pool.tile([C, C], f32r)
            nc.vector.tensor_copy(ones_cc[:], ones_f32[:])

            # PE warmup: the HAM clock gate keeps the PE at 1.2 GHz until
            # it has been busy for a full ~3.4us activity window. The real
            # stream can't start until x/w DMAs land (~4us away), so burn
            # that wait on matmuls with no DMA deps: the PE hits 2.4 GHz
            # right as the first real matmul issues, instead of running
            # the first ~8us of layer 1 at half clock. bf16 operands so
            # the dedup pass collapses the weight reloads.
            warm_w = const_pool.tile([C, C], bf16)
            nc.vector.tensor_copy(warm_w[:], ones_f32[:])
            warm_in = const_pool.tile([C, 512], bf16)
            nc.gpsimd.memset(warm_in[:], 0.0)
            for _ in range(8):
                pwarm = pb_pool.tile([C, 512], f32, tag="pb", name="pwarm")
                nc.tensor.matmul(
                    pwarm[:], warm_w[:], warm_in[:], start=True, stop=True
                )

            # x on the SP ring; chunk 0 in two halves so layer-1 m=0 can
            # start on k=0..3 while k=4..7 is in flight. Triggers are
            # STAGGERED post-schedule (see _stagger_x_dmas): the 16 SDMA
            # engines round-robin descriptors of every in-flight transfer,
            # so if all 6 chunks are triggered at once, chunk 0 gets ~1/6
            # of the bandwidth and only completes near the end of the
            # whole 4.5MB stream — the PE then idles ~7us waiting for its
            # first columns.
            xt = []
            for ci, cn in enumerate(chunks):
                t = x_pool.tile([128, K1, cn], bf16, tag="xt", name="xt")
                if ci == 0:
                    nc.sync.dma_start(t[:, : K1 // 2, :], x_d[ci][:, : K1 // 2, :])
                    nc.sync.dma_start(t[:, K1 // 2 :, :], x_d[ci][:, K1 // 2 :, :])
                else:
                    nc.sync.dma_start(t[:], x_d[ci][:])
                xt.append(t)

            # ---- Layer 1: weights-outer, chunks-inner per group ---------
            # Each (m, k) weight tile is loaded once per group; the dedup
            # pass removes the redundant per-matmul reloads. Accumulation
            # groups for the chunks of a group interleave across PSUM
            # banks, which is fine on hardware (skip the sim's group
            # check).
            ht = {}  # (m, ci) -> bf16 SBUF tile

            def l1_group(grp):
                ph = {}
                for m in range(M1):
                    for k in range(K1):
                        for ci in grp:
                            if k == 0:
                                ph[ci] = ph_pool.tile(
                                    [128, chunks[ci]], f32, tag="ph", name="ph"
                                )
                            nc.tensor.matmul(
                                ph[ci][:],
                                w1_sb[:, m, k, :],
                                xt[ci][:, k, :],
                                start=(k == 0),
                                stop=(k == K1 - 1),
                                skip_group_check=True,
                            )
                            if k == K1 - 1:
                                hm = h_pool.tile(
                                    [128, chunks[ci]], bf16, tag="ht"
                                )
                                nc.scalar.activation(
                                    hm[:],
                                    ph[ci][:],
                                    mybir.ActivationFunctionType.Relu,
                                    bias=b1_sb[:, ds(m, 1)],
                                )
                                ht[(m, ci)] = hm

            # ---- Layer 2 + softmax, chunk-wise --------------------------
            def l2_body(ci: int):
                cn = chunks[ci]
                pl = pl_pool.tile([C, cn], f32, tag="pl")
                for k in range(K2):
                    nc.tensor.matmul(
                        pl[:],
                        w2_sb[:, k, :],
                        ht[(k, ci)][:],
                        start=(k == 0),
                        stop=(k == K2 - 1),
                    )
                expt = e_pool.tile([C, cn], f32r, tag="expt")
                nc.scalar.activation(
                    expt[:],
                    pl[:],
                    mybir.ActivationFunctionType.Exp,
                    bias=b2_sb[:, 0:1],
                )
                return expt

            def tail(expt, ci: int):
                """Softmax normalization + store for one chunk."""
                cn = chunks[ci]
                pb = pb_pool.tile([C, cn], f32, tag="pb")
                nc.tensor.matmul(pb[:], ones_cc[:], expt[:], start=True, stop=True)
                rec = r_pool.tile([C, cn], f32, tag="rec")
                nc.vector.reciprocal_approx_fast(rec[:], pb[:])
                ot = o_pool.tile([C, cn], f32, tag="ot")
                nc.vector.tensor_mul(ot[:], expt[:].bitcast(f32), rec[:])
                # GpSimd SWDGE: a store that waits on the softmax chain
                # would stall later x loads (sync ring) or relu/exp issue
                # (scalar ring); the gpsimd queue is otherwise idle.
                nc.gpsimd.dma_start(out_d[:, ds(n0s[ci], cn)], ot[:])

            # Interleave: each group's layer 2 is emitted after the NEXT
            # group's layer 1, so only the last group's layer 2 sits past
            # the end of layer 1 (shorter exposed tail), and the pending
            # softmax tails stay one chunk behind the bodies so each
            # softmax-sum matmul overlaps later PE work instead of
            # stalling on the ACT exp.
            pending = []

            def l2_batch(chunk_ids, final=False):
                for ci in chunk_ids:
                    expt = l2_body(ci)
                    if pending:
                        tail(*pending.pop(0))
                    pending.append((expt, ci))
                if final:
                    while pending:
                        tail(*pending.pop(0))

            for gi, grp in enumerate(groups):
                l1_group(grp)
                if gi > 0:
                    l2_batch(groups[gi - 1])
            l2_batch(groups[-1], final=True)

    # Layer-1 matmuls past the first of their (m, k, group) run should lose
    # their reload. The tile scheduler occasionally interleaves other PE
    # work and keeps a few extra loads; require at least half to confirm
    # the weight-reuse structure survived scheduling at all.
    n_removed = _dedup_ldweights(nc)
    n_l1_dups = sum(K1 * M1 * (len(grp) - 1) for grp in groups)
    assert n_removed >= n_l1_dups // 2, (n_removed, n_l1_dups)

    # Each layer-1 group's x chunks get the full DMA bandwidth until they
    # land, in consumption order.
    gates = {}
    for gi in range(1, len(groups)):
        first = groups[gi][0]
        gates[f"xt{first}"] = [f"xt{c}" for c in groups[gi - 1]]
    n_gates = _stagger_x_dmas(nc, gates)
    assert n_gates >= len(gates), (n_gates, gates)

    nc.compile()
    _program_cache[key] = nc
    return nc


def kernel(domain, x, W1, b1, W2, b2):
    domain = np.asarray(domain)
    x = np.asarray(x, dtype=np.float32)
    W1 = np.asarray(W1, dtype=np.float32)
    b1 = np.asarray(b1, dtype=np.float32)
    W2 = np.asarray(W2, dtype=np.float32)
    b2 = np.asarray(b2, dtype=np.float32)

    B, F1 = x.shape
    E, _, F2 = W1.shape
    C = W2.shape[2]
    K1 = F1 // 128
    K2 = F2 // 128
    assert E == N_CORES

    idx = [np.nonzero(domain == e)[0] for e in range(E)]
    counts = [len(i) for i in idx]
    cap = max(512, int(math.ceil(max(counts) / 128)) * 128)
    chunks = _chunk_sizes(cap)

    nc = _build_program(cap, F1, F2, C)

    x_bf = x.astype(bf16_np)
    W1_bf = W1.astype(bf16_np)
    W2_bf = W2.astype(bf16_np)

    in_maps = []
    for e in range(E):
        xT = np.zeros((F1, cap), bf16_np)
        xT[:, : counts[e]] = x_bf[idx[e]].T
        # [F1, cap] -> per-chunk [128, K1, cn] blocks (SBUF tile layout).
        xT3 = xT.reshape(K1, 128, cap)
        m = {
            "w1": np.ascontiguousarray(
                W1_bf[e].reshape(K1, 128, F2 // 128, 128).transpose(1, 2, 0, 3)
            ),
            "b1t": np.ascontiguousarray(b1[e].reshape(F2 // 128, 128).T),
            "w2": np.ascontiguousarray(
                W2_bf[e].reshape(K2, 128, C).transpose(1, 0, 2)
            ),
            "b2t": np.ascontiguousarray(b2[e].reshape(C, 1)),
        }
        n0 = 0
        for ci, cn in enumerate(chunks):
            m[f"xt{ci}"] = np.ascontiguousarray(
                xT3[:, :, n0 : n0 + cn].transpose(1, 0, 2)
            )
            n0 += cn
        in_maps.append(m)

    res = run_bass_kernel_spmd(nc, in_maps, core_ids=list(range(N_CORES)))

    out = np.empty((B, C), np.float32)
    for e in range(E):
        out[idx[e]] = res.results[e]["outT"][:, : counts[e]].T
    return out
